# revision 7
# baseline (speedup 1.0000x reference)
"""Trainium2 Bass kernel for DeformableSincConv1d (v3, matmul-shift design).

Data parallel over batch: 4 rows/core on 8 cores. Per core, per batch-pair:
  1. Windowed im2col load (fp16): xx[p, c, j] = x[1280c + 10p + j]; row 0 of
     the pair at cols 0..52, row 1 at cols 64..116 (junk cols 53..63 zeroed).
  2. One PE transpose per 128-l chunk over cols 0..116 -> psum [117, 128];
     a single Act/DVE copy evacuates psum[0:115] straight into X0P — no
     SBUF->SBUF DMA at all (x is laid out with no +1 pad so j aligns).
  3. Sampling, all partition shifts as matmuls: qp = wr2^T X0P (offset conv),
     ep = Emat^T X0P (= X0P[j+1]-X0P[j], col 50 zero), em = Mneg^T X0P
     (= X0P[j-1]-X0P[j], col 0 zero). Act computes o+ = relu(qp + b) and
     o- = relu(-qp - b); deformed = X0P + o+*ep + o-*em exactly reproduces
     the mask/select lerp (offsets |o| < 1).  dd tiles are l-major [51, 3200]
     so every element-wise op is contiguous.
  4. Final conv: stacked rotated-filter decomposition, A-half rows 0..50 and
     B-half rows 64..114 of f128; per t0 two matmuls accumulate in psum
     (rhs = dd strided plane views l = a + 10s and l = a+1 + 10s), so the
     dd[51:102] plane-rotation copies are gone. Psum evacuated with the
     (s, t0)-pair interleaved scatter into fp16 ysb; y written to HBM fp16.
"""

import sys

import numpy as np

if "/opt/trn_rl_repo" not in sys.path:
    sys.path.insert(0, "/opt/trn_rl_repo")

SR = 16000
C_OUT = 80
K = 51
STRIDE = 10
HALF = (K - 1) // 2

B_FULL = 32
N_CORES = 8
B_LOC = B_FULL // N_CORES
L_FULL = 32000

R1 = 64          # partition base of second row in pair tiles
NP = R1 + K      # 115 rows in pair tiles


def _derive(L):
    L_out = (L - K) // STRIDE + 1
    T_out = (L_out * K - K) // STRIDE + 1
    NCHUNK = (L_out + 127) // 128
    LPAD = NCHUNK * 128
    XLEN = 10 * LPAD + 48
    return L_out, T_out, NCHUNK, LPAD, XLEN


def _host_filters(hz, band):
    hzc = np.clip(hz.astype(np.float32), 0.0, SR / 2).astype(np.float32)
    bandc = np.clip(band.astype(np.float32), 3.0, SR / 2).astype(np.float32)
    t_right = (np.arange(1, HALF + 1, dtype=np.float32) / np.float32(SR)).astype(np.float32)
    low = (hzc - bandc / 2).astype(np.float32)
    high = (hzc + bandc / 2).astype(np.float32)

    def sinc(t):
        ts = np.where(t == 0, np.float32(1.0), t)
        return np.where(t == 0, np.float32(1.0), np.sin(ts) / ts).astype(np.float32)

    a1 = (2 * high).astype(np.float32)
    a2 = (2 * low).astype(np.float32)
    bp_left = (a1 * sinc(a1 * t_right) - a2 * sinc(a2 * t_right)).astype(np.float32)
    bp = np.concatenate([bp_left, np.ones((C_OUT, 1), np.float32), bp_left[:, ::-1]], axis=1)
    return (bp / (2 * bandc)).astype(np.float32)  # [C_OUT, K]


def _host_f128(filt, L):
    """Stacked rotated filter matrices [128, K*C_OUT]; rows 0..50 = A-half
    (column offset a), rows 64..114 = B-half (column offset a+1)."""
    L_out, T_out, _, LPAD, _ = _derive(L)
    F = np.zeros((128, K, C_OUT), np.float32)
    for t0 in range(K):
        a = (STRIDE * t0) // K
        ns = (T_out - 1 - t0) // K + 1
        for k2 in range(K):
            kstar = (k2 + STRIDE * t0) % K
            lstar = (STRIDE * t0 + k2) // K
            if lstar == a:
                F[kstar, t0, :] = filt[:, k2]
            else:
                assert lstar == a + 1
                F[R1 + kstar, t0, :] = filt[:, k2]
        assert a + 1 + STRIDE * (ns - 1) <= L_out - 1
    return F.reshape(128, K * C_OUT)


def _host_shift_mats():
    """emat: col j -> X0P[j+1]-X0P[j] (j<50), col 50 zero.
    mmatn: col j -> X0P[j-1]-X0P[j] (j>0), col 0 zero.  Block-diag at 0, R1."""
    emat = np.zeros((NP, NP), np.float32)
    mmatn = np.zeros((NP, NP), np.float32)
    for base in (0, R1):
        for j in range(K - 1):
            emat[base + j + 1, base + j] = 1.0
            emat[base + j, base + j] = -1.0
        for j in range(1, K):
            mmatn[base + j - 1, base + j] = 1.0
            mmatn[base + j, base + j] = -1.0
    return emat, mmatn


def build_program(B_loc=B_LOC, L=L_FULL, debug=False):
    import concourse.bacc as bacc
    import concourse.tile as tile
    from concourse import bass, mybir

    f32 = mybir.dt.float32
    f16 = mybir.dt.float16
    Act = mybir.ActivationFunctionType

    L_out, T_out, NCHUNK, LPAD, XLEN = _derive(L)
    NSMAX = (T_out - 1) // K + 1    # 320
    LSAMP = STRIDE * NSMAX          # sampled deformed region (3200)
    assert LSAMP <= LPAD
    NG = (NCHUNK + 7) // 8          # transpose psum groups of 8 chunks
    CC = 510
    NCC = (LSAMP + CC - 1) // CC
    n_pairs = B_loc // 2
    assert n_pairs == 2

    nc = bacc.Bacc("TRN2", target_bir_lowering=False, debug=debug)

    x_d = nc.dram_tensor("x", [B_loc, XLEN], f16, kind="ExternalInput")
    wr2_d = nc.dram_tensor("wr2", [NP, NP], f16, kind="ExternalInput")
    emat_d = nc.dram_tensor("emat", [NP, NP], f16, kind="ExternalInput")
    mmatn_d = nc.dram_tensor("mmatn", [NP, NP], f16, kind="ExternalInput")
    offb2_d = nc.dram_tensor("offb2", [NP, 1], f32, kind="ExternalInput")
    negoffb2_d = nc.dram_tensor("negoffb2", [NP, 1], f32, kind="ExternalInput")
    fa_d = nc.dram_tensor("fa", [K, K * C_OUT], f16, kind="ExternalInput")
    fb_d = nc.dram_tensor("fb", [K, K * C_OUT], f16, kind="ExternalInput")
    ident_d = nc.dram_tensor("ident", [128, 128], f16, kind="ExternalInput")
    y_d = nc.dram_tensor("y", [B_loc, C_OUT, T_out], f16, kind="ExternalOutput")

    xap = x_d[:]

    with tile.TileContext(nc) as tc:
        with (
            tc.tile_pool(name="consts", bufs=1) as consts,
            tc.tile_pool(name="xxp", bufs=2) as xxp,
            tc.tile_pool(name="x0p", bufs=2) as x0p,
            tc.tile_pool(name="qsp", bufs=4) as qsp,
            tc.tile_pool(name="emp", bufs=6) as emp,
            tc.tile_pool(name="ddp", bufs=4) as ddp,
            tc.tile_pool(name="ysbp", bufs=2) as ysbp,
            tc.tile_pool(name="tpsum", bufs=2, space="PSUM") as tpsum,
            tc.tile_pool(name="qpsum", bufs=4, space="PSUM") as qpsum,
            tc.tile_pool(name="fpsum", bufs=2, space="PSUM") as fpsum,
        ):
            wr2_sb = consts.tile([NP, NP], f16)
            nc.sync.dma_start(out=wr2_sb[:], in_=wr2_d[:])
            emat_sb = consts.tile([NP, NP], f16)
            nc.sync.dma_start(out=emat_sb[:], in_=emat_d[:])
            mmatn_sb = consts.tile([NP, NP], f16)
            nc.sync.dma_start(out=mmatn_sb[:], in_=mmatn_d[:])
            offb2_sb = consts.tile([NP, 1], f32)
            nc.sync.dma_start(out=offb2_sb[:], in_=offb2_d[:])
            negoffb2_sb = consts.tile([NP, 1], f32)
            nc.sync.dma_start(out=negoffb2_sb[:], in_=negoffb2_d[:])
            fa_sb = consts.tile([K, K * C_OUT], f16)
            nc.sync.dma_start(out=fa_sb[:], in_=fa_d[:])
            fb_sb = consts.tile([K, K * C_OUT], f16)
            nc.sync.dma_start(out=fb_sb[:], in_=fb_d[:])
            ident_sb = consts.tile([128, 128], f16)
            nc.sync.dma_start(out=ident_sb[:], in_=ident_d[:])

            def ecopy(eng, dst, src):
                if eng is nc.scalar:
                    eng.copy(dst, src)
                else:
                    eng.tensor_copy(dst, src)

            def load_pair_dma(p):
                xx = xxp.tile([128, NCHUNK, 128], f16, tag="xx")
                nc.vector.memset(xx[:, :, 53:64], 0.0)
                for ri in range(2):
                    col0 = R1 * ri
                    for g in range(NG):
                        h0 = 8 * g
                        hn = min(8, NCHUNK - h0)
                        in_ap = bass.AP(
                            tensor=xap.tensor,
                            offset=(2 * p + ri) * XLEN + 1280 * h0,
                            ap=[[10, 128], [1280, hn], [1, 53]],
                        )
                        nc.sync.dma_start(out=xx[:, h0:h0 + hn, col0:col0 + 53],
                                          in_=in_ap)
                return xx

            def load_pair_tp(p, xx, X0P):
                for g in range(NG):
                    n = min(8, NCHUNK - 8 * g)
                    pt = tpsum.tile([117, 1024], f16, tag="pt")
                    for c in range(n):
                        nc.tensor.transpose(pt[:, c * 128:(c + 1) * 128],
                                            xx[:, 8 * g + c, 0:117], ident_sb[:])
                    lo = g * 1024
                    nw = min(n * 128, LSAMP - lo)
                    if nw > 0:
                        ecopy([nc.vector, nc.scalar][(p + g) % 2],
                              X0P[:, lo:lo + nw], pt[0:NP, :nw])

            def alloc_pair():
                X0P = x0p.tile([NP, LPAD], f16)
                dd0 = ddp.tile([K, LSAMP], f16, tag="dd")
                dd1 = ddp.tile([K, LSAMP], f16, tag="dd")
                return X0P, dd0, dd1

            def front_chunk(state, c7):
                """Sampling chain for one 510-column l-chunk; contiguous ops."""
                X0P, dd0, dd1 = state
                n = min(CC, LSAMP - c7 * CC)
                sl = slice(c7 * CC, c7 * CC + n)
                qp = qpsum.tile([NP, CC], f32, tag="qps")
                nc.tensor.matmul(qp[:, :n], wr2_sb[:], X0P[:, sl],
                                 start=True, stop=True)
                ep = qpsum.tile([NP, CC], f32, tag="qps")
                nc.tensor.matmul(ep[:, :n], emat_sb[:], X0P[:, sl],
                                 start=True, stop=True)
                em = qpsum.tile([NP, CC], f32, tag="qps")
                nc.tensor.matmul(em[:, :n], mmatn_sb[:], X0P[:, sl],
                                 start=True, stop=True)
                QSP = qsp.tile([NP, CC], f16, tag="qs")
                nc.scalar.activation(QSP[:, :n], qp[:, :n], Act.Relu,
                                     bias=offb2_sb[:], scale=1.0)
                QSM = qsp.tile([NP, CC], f16, tag="qs")
                nc.scalar.activation(QSM[:, :n], qp[:, :n], Act.Relu,
                                     bias=negoffb2_sb[:], scale=-1.0)
                mA = emp.tile([NP, CC], f16, tag="em")
                nc.vector.tensor_mul(mA[:, :n], ep[:, :n], QSP[:, :n])
                mB = emp.tile([NP, CC], f16, tag="em")
                nc.vector.tensor_mul(mB[:, :n], em[:, :n], QSM[:, :n])
                S = emp.tile([NP, CC], f16, tag="em")
                nc.gpsimd.tensor_add(S[:, :n], mA[:, :n], mB[:, :n])
                nc.gpsimd.tensor_add(dd0[:, sl], X0P[0:K, sl], S[0:K, :n])
                eng1 = nc.vector if c7 % 2 else nc.gpsimd
                eng1.tensor_add(dd1[:, sl], X0P[R1:NP, sl], S[R1:NP, :n])

            SCATTER = {0: nc.scalar, 1: nc.vector}
            HH = NSMAX // 2   # s-half size (160)

            def fmm(fp_slice, dd, t0, s_lo, n):
                a = (STRIDE * t0) // K
                t0C = slice(t0 * C_OUT, (t0 + 1) * C_OUT)
                a0 = a + STRIDE * s_lo
                b0 = a + 1 + STRIDE * s_lo
                rhsA = dd[0:K, a0:a0 + STRIDE * (n - 1) + 1:STRIDE]
                rhsB = dd[0:K, b0:b0 + STRIDE * (n - 1) + 1:STRIDE]
                nc.tensor.matmul(fp_slice, fa_sb[0:K, t0C], rhsA,
                                 start=True, stop=False)
                nc.tensor.matmul(fp_slice, fb_sb[0:K, t0C], rhsB,
                                 start=False, stop=True)

            def final_pair_h(r, dd, ysb, t0, h):
                """t0 and t0+1 interleaved in one psum bank per s-half; the
                evacuation writes (s, t0)-pairs so consecutive stores are
                4-byte adjacent in ysb."""
                ns0 = (T_out - 1 - t0) // K + 1
                ns1 = (T_out - 1 - (t0 + 1)) // K + 1
                s_lo = h * HH
                n0 = min(ns0 - s_lo, HH)
                n1 = min(ns1 - s_lo, HH)
                fp = fpsum.tile([C_OUT, 2, HH], f32, tag="fp")
                fmm(fp[:, 0, :n0], dd, t0, s_lo, n0)
                fmm(fp[:, 1, :n1], dd, t0 + 1, s_lo, n1)
                base = t0 + K * s_lo
                yv2 = ysb[:, base:base + K * n1].rearrange(
                    "p (s q) -> p s q", q=K)[:, :, 0:2]
                sv = fp[:, :, :n1].rearrange("p t s -> p s t")
                ecopy(SCATTER[(r + t0 + h) % 2], yv2, sv)
                if n0 > n1:
                    yt = ysb[:, base + K * n1:base + K * n1 + 1]
                    ecopy(SCATTER[(r + t0 + h + 1) % 2], yt, fp[:, 0, n1:n0])

            def final_last_h(r, dd, ysb, h):
                t0 = K - 1
                ns = (T_out - 1 - t0) // K + 1
                s_lo = h * HH
                n = min(ns - s_lo, HH)
                fp = fpsum.tile([C_OUT, 2, HH], f32, tag="fp")
                fmm(fp[:, 0, :n], dd, t0, s_lo, n)
                yv = ysb[:, t0 + K * s_lo:t0 + K * (s_lo + n - 1) + 1:K]
                ecopy(SCATTER[(r + h) % 2], yv, fp[:, 0, :n])

            def final_rowpass(r, dd, ysb, front=None):
                """front: optional list of thunks interleaved between t0-pairs
                (pair-1 sampling chunks issued during row-0 final)."""
                i = 0
                for t0 in range(0, K - 1, 2):
                    for h in range(2):
                        final_pair_h(r, dd, ysb, t0, h)
                    if front is not None and t0 % 8 == 0 and i < len(front):
                        front[i]()
                        i += 1
                for h in range(2):
                    final_last_h(r, dd, ysb, h)
                if front is not None:
                    while i < len(front):
                        front[i]()
                        i += 1

            YSBW = T_out + K - 1

            st0 = alloc_pair()
            st1 = alloc_pair()
            xx0 = load_pair_dma(0)
            load_pair_tp(0, xx0, st0[0])
            xx1 = load_pair_dma(1)
            for c7 in range(NCC):
                front_chunk(st0, c7)
            load_pair_tp(1, xx1, st1[0])

            ysb0 = ysbp.tile([C_OUT, YSBW], f16, tag="ysb")
            front1 = [
                (lambda i=i: front_chunk(st1, i)) for i in range(NCC)
            ]
            final_rowpass(0, st0[1], ysb0, front=front1)
            nc.sync.dma_start(out=y_d[0], in_=ysb0[:, :T_out])

            ysb1 = ysbp.tile([C_OUT, YSBW], f16, tag="ysb")
            final_rowpass(1, st0[2], ysb1)
            nc.sync.dma_start(out=y_d[1], in_=ysb1[:, :T_out])

            ysb2 = ysbp.tile([C_OUT, YSBW], f16, tag="ysb")
            final_rowpass(2, st1[1], ysb2)
            nc.sync.dma_start(out=y_d[2], in_=ysb2[:, :T_out])

            ysb3 = ysbp.tile([C_OUT, YSBW], f16, tag="ysb")
            final_rowpass(3, st1[2], ysb3)
            nc.sync.dma_start(out=y_d[3], in_=ysb3[:, :T_out])

    nc.compile()
    return nc


def _host_inputs(x, hz, band, offset_w, offset_b, B_loc, L):
    """Build the per-core input maps."""
    L_out, T_out, NCHUNK, LPAD, XLEN = _derive(L)
    filt = _host_filters(hz, band)
    f128 = _host_f128(filt, L).astype(np.float16)
    fa = np.ascontiguousarray(f128[0:K])
    fb = np.ascontiguousarray(f128[R1:R1 + K])
    wr = offset_w[:, 0, :].T.astype(np.float32)  # [k_in, k_out]
    wr2 = np.zeros((NP, NP), np.float32)
    wr2[0:K, 0:K] = wr
    wr2[R1:NP, R1:NP] = wr
    emat, mmatn = _host_shift_mats()
    offb2 = np.zeros((NP, 1), np.float32)
    offb2[0:K, 0] = offset_b.astype(np.float32)
    offb2[R1:NP, 0] = offset_b.astype(np.float32)
    negoffb2 = -offb2
    ident = np.eye(128, dtype=np.float16)

    B = x.shape[0]
    xpad = np.zeros((B, XLEN), np.float16)
    xpad[:, 0:L] = x.astype(np.float16)

    n_cores = B // B_loc
    in_maps = []
    for i in range(n_cores):
        in_maps.append({
            "x": np.ascontiguousarray(xpad[i * B_loc:(i + 1) * B_loc]),
            "wr2": wr2.astype(np.float16),
            "emat": emat.astype(np.float16),
            "mmatn": mmatn.astype(np.float16),
            "offb2": offb2,
            "negoffb2": negoffb2,
            "fa": fa,
            "fb": fb,
            "ident": ident,
        })
    return in_maps


_CACHED = {}


def _get_program():
    key = (B_LOC, L_FULL)
    if key not in _CACHED:
        _CACHED[key] = build_program(B_LOC, L_FULL)
    return _CACHED[key]


def kernel(x, hz, band, offset_w, offset_b):
    from concourse.bass_utils import run_bass_kernel_spmd

    x = np.asarray(x, dtype=np.float32)
    hz = np.asarray(hz, dtype=np.float32)
    band = np.asarray(band, dtype=np.float32)
    offset_w = np.asarray(offset_w, dtype=np.float32)
    offset_b = np.asarray(offset_b, dtype=np.float32)

    nc = _get_program()
    in_maps = _host_inputs(x, hz, band, offset_w, offset_b, B_LOC, L_FULL)
    res = run_bass_kernel_spmd(nc, in_maps, list(range(N_CORES)))
    outs = [res.results[i]["y"] for i in range(N_CORES)]
    return np.concatenate(outs, axis=0).astype(np.float32)


# revision 9
# speedup vs baseline: 1.3372x; 1.3372x over previous
"""Trainium2 Bass kernel for DeformableSincConv1d (v3, matmul-shift design).

Data parallel over batch: 4 rows/core on 8 cores. Per core, per batch-pair:
  1. Windowed im2col load (fp16): xx[p, c, j] = x[1280c + 10p + j]; row 0 of
     the pair at cols 0..52, row 1 at cols 64..116 (junk cols 53..63 zeroed).
  2. One PE transpose per 128-l chunk over cols 0..116 -> psum [117, 128];
     a single Act/DVE copy evacuates psum[0:115] straight into X0P — no
     SBUF->SBUF DMA at all (x is laid out with no +1 pad so j aligns).
  3. Sampling, all partition shifts as matmuls: qp = wr2^T X0P (offset conv),
     ep = Emat^T X0P (= X0P[j+1]-X0P[j], col 50 zero), em = Mneg^T X0P
     (= X0P[j-1]-X0P[j], col 0 zero). Act computes o+ = relu(qp + b) and
     o- = relu(-qp - b); deformed = X0P + o+*ep + o-*em exactly reproduces
     the mask/select lerp (offsets |o| < 1).  dd tiles are l-major [51, 3200]
     so every element-wise op is contiguous.
  4. Final conv: stacked rotated-filter decomposition, A-half rows 0..50 and
     B-half rows 64..114 of f128; per t0 two matmuls accumulate in psum
     (rhs = dd strided plane views l = a + 10s and l = a+1 + 10s), so the
     dd[51:102] plane-rotation copies are gone. Psum evacuated with the
     (s, t0)-pair interleaved scatter into fp16 ysb; y written to HBM fp16.
"""

import sys

import numpy as np

if "/opt/trn_rl_repo" not in sys.path:
    sys.path.insert(0, "/opt/trn_rl_repo")

SR = 16000
C_OUT = 80
K = 51
STRIDE = 10
HALF = (K - 1) // 2

B_FULL = 32
N_CORES = 8
B_LOC = B_FULL // N_CORES
L_FULL = 32000

R1 = 64          # partition base of second row in pair tiles
NP = R1 + K      # 115 rows in pair tiles


def _derive(L):
    L_out = (L - K) // STRIDE + 1
    T_out = (L_out * K - K) // STRIDE + 1
    NCHUNK = (L_out + 127) // 128
    LPAD = NCHUNK * 128
    XLEN = 10 * LPAD + 48
    return L_out, T_out, NCHUNK, LPAD, XLEN


def _host_filters(hz, band):
    hzc = np.clip(hz.astype(np.float32), 0.0, SR / 2).astype(np.float32)
    bandc = np.clip(band.astype(np.float32), 3.0, SR / 2).astype(np.float32)
    t_right = (np.arange(1, HALF + 1, dtype=np.float32) / np.float32(SR)).astype(np.float32)
    low = (hzc - bandc / 2).astype(np.float32)
    high = (hzc + bandc / 2).astype(np.float32)

    def sinc(t):
        ts = np.where(t == 0, np.float32(1.0), t)
        return np.where(t == 0, np.float32(1.0), np.sin(ts) / ts).astype(np.float32)

    a1 = (2 * high).astype(np.float32)
    a2 = (2 * low).astype(np.float32)
    bp_left = (a1 * sinc(a1 * t_right) - a2 * sinc(a2 * t_right)).astype(np.float32)
    bp = np.concatenate([bp_left, np.ones((C_OUT, 1), np.float32), bp_left[:, ::-1]], axis=1)
    return (bp / (2 * bandc)).astype(np.float32)  # [C_OUT, K]


def _host_f128(filt, L):
    """Stacked rotated filter matrices [128, K*C_OUT]; rows 0..50 = A-half
    (column offset a), rows 64..114 = B-half (column offset a+1)."""
    L_out, T_out, _, LPAD, _ = _derive(L)
    F = np.zeros((128, K, C_OUT), np.float32)
    for t0 in range(K):
        a = (STRIDE * t0) // K
        ns = (T_out - 1 - t0) // K + 1
        for k2 in range(K):
            kstar = (k2 + STRIDE * t0) % K
            lstar = (STRIDE * t0 + k2) // K
            if lstar == a:
                F[kstar, t0, :] = filt[:, k2]
            else:
                assert lstar == a + 1
                F[R1 + kstar, t0, :] = filt[:, k2]
        assert a + 1 + STRIDE * (ns - 1) <= L_out - 1
    return F.reshape(128, K * C_OUT)


def _host_shift_mats():
    """emat: col j -> X0P[j+1]-X0P[j] (j<50), col 50 zero.
    mmatn: col j -> X0P[j-1]-X0P[j] (j>0), col 0 zero.  Block-diag at 0, R1."""
    emat = np.zeros((NP, NP), np.float32)
    mmatn = np.zeros((NP, NP), np.float32)
    for base in (0, R1):
        for j in range(K - 1):
            emat[base + j + 1, base + j] = 1.0
            emat[base + j, base + j] = -1.0
        for j in range(1, K):
            mmatn[base + j - 1, base + j] = 1.0
            mmatn[base + j, base + j] = -1.0
    return emat, mmatn


def build_program(B_loc=B_LOC, L=L_FULL, debug=False):
    import concourse.bacc as bacc
    import concourse.tile as tile
    from concourse import bass, mybir

    f32 = mybir.dt.float32
    f16 = mybir.dt.float16
    Act = mybir.ActivationFunctionType

    L_out, T_out, NCHUNK, LPAD, XLEN = _derive(L)
    NSMAX = (T_out - 1) // K + 1    # 320
    LSAMP = STRIDE * NSMAX          # sampled deformed region (3200)
    assert LSAMP <= LPAD
    NG = (NCHUNK + 7) // 8          # transpose psum groups of 8 chunks
    CC = 510
    NCC = (LSAMP + CC - 1) // CC
    n_pairs = B_loc // 2
    assert n_pairs == 2

    nc = bacc.Bacc("TRN2", target_bir_lowering=False, debug=debug)

    x_d = nc.dram_tensor("x", [B_loc, XLEN], f16, kind="ExternalInput")
    wr2_d = nc.dram_tensor("wr2", [NP, NP], f16, kind="ExternalInput")
    emat_d = nc.dram_tensor("emat", [NP, NP], f16, kind="ExternalInput")
    mmatn_d = nc.dram_tensor("mmatn", [NP, NP], f16, kind="ExternalInput")
    offb2_d = nc.dram_tensor("offb2", [NP, 1], f32, kind="ExternalInput")
    negoffb2_d = nc.dram_tensor("negoffb2", [NP, 1], f32, kind="ExternalInput")
    fa_d = nc.dram_tensor("fa", [K, K * C_OUT], f16, kind="ExternalInput")
    fb_d = nc.dram_tensor("fb", [K, K * C_OUT], f16, kind="ExternalInput")
    ident_d = nc.dram_tensor("ident", [128, 128], f16, kind="ExternalInput")
    y_d = nc.dram_tensor("y", [B_loc, C_OUT, T_out], f16, kind="ExternalOutput")

    xap = x_d[:]

    with tile.TileContext(nc) as tc:
        with (
            tc.tile_pool(name="consts", bufs=1) as consts,
            tc.tile_pool(name="xxp", bufs=2) as xxp,
            tc.tile_pool(name="x0p", bufs=2) as x0p,
            tc.tile_pool(name="qsp", bufs=4) as qsp,
            tc.tile_pool(name="emp", bufs=6) as emp,
            tc.tile_pool(name="ddp", bufs=4) as ddp,
            tc.tile_pool(name="ysbp", bufs=2) as ysbp,
            tc.tile_pool(name="tpsum", bufs=1, space="PSUM") as tpsum,
            tc.tile_pool(name="qpsum", bufs=3, space="PSUM") as qpsum,
            tc.tile_pool(name="fpsum", bufs=2, space="PSUM") as fpsum,
        ):
            wr2_sb = consts.tile([NP, NP], f16)
            nc.sync.dma_start(out=wr2_sb[:], in_=wr2_d[:])
            emat_sb = consts.tile([NP, NP], f16)
            nc.sync.dma_start(out=emat_sb[:], in_=emat_d[:])
            mmatn_sb = consts.tile([NP, NP], f16)
            nc.sync.dma_start(out=mmatn_sb[:], in_=mmatn_d[:])
            offb2_sb = consts.tile([NP, 1], f32)
            nc.sync.dma_start(out=offb2_sb[:], in_=offb2_d[:])
            negoffb2_sb = consts.tile([NP, 1], f32)
            nc.sync.dma_start(out=negoffb2_sb[:], in_=negoffb2_d[:])
            fa_sb = consts.tile([K, K * C_OUT], f16)
            nc.sync.dma_start(out=fa_sb[:], in_=fa_d[:])
            fb_sb = consts.tile([K, K * C_OUT], f16)
            nc.sync.dma_start(out=fb_sb[:], in_=fb_d[:])
            ident_sb = consts.tile([128, 128], f16)
            nc.sync.dma_start(out=ident_sb[:], in_=ident_d[:])

            def ecopy(eng, dst, src):
                if eng is nc.scalar:
                    eng.copy(dst, src)
                else:
                    eng.tensor_copy(dst, src)

            def load_pair_dma(p):
                """im2col in polyphase window order: xx partition pi%128 of
                chunk pi//128 holds the window for pi = 320*r + s, i.e.
                l = 10*s + r, so downstream X0P/dd are plane-major and every
                op (including final-conv rhs) is contiguous."""
                xx = xxp.tile([128, NCHUNK, 128], f16, tag="xx")
                nc.vector.memset(xx[:, :, 53:64], 0.0)
                for ri in range(2):
                    col0 = R1 * ri
                    row_off = (2 * p + ri) * XLEN
                    for c in range(NCHUNK):
                        pi = 128 * c
                        while pi < 128 * (c + 1):
                            r, s0 = divmod(pi, NSMAX)
                            npart = min(128 * (c + 1) - pi, NSMAX - s0)
                            in_ap = bass.AP(
                                tensor=xap.tensor,
                                offset=row_off + 100 * s0 + 10 * r,
                                ap=[[100, npart], [1, 53]],
                            )
                            p0 = pi - 128 * c
                            nc.sync.dma_start(
                                out=xx[p0:p0 + npart, c, col0:col0 + 53],
                                in_=in_ap)
                            pi += npart
                return xx

            def load_pair_tp(p, xx, X0P):
                for g in range(NG):
                    n = min(8, NCHUNK - 8 * g)
                    pt = tpsum.tile([117, 1024], f16, tag="pt")
                    for c in range(n):
                        nc.tensor.transpose(pt[:, c * 128:(c + 1) * 128],
                                            xx[:, 8 * g + c, 0:117], ident_sb[:])
                    lo = g * 1024
                    nw = min(n * 128, LSAMP - lo)
                    if nw > 0:
                        ecopy([nc.vector, nc.scalar][(p + g) % 2],
                              X0P[:, lo:lo + nw], pt[0:NP, :nw])

            def alloc_pair():
                X0P = x0p.tile([NP, LPAD], f16)
                dd0 = ddp.tile([K, LSAMP], f16, tag="dd")
                dd1 = ddp.tile([K, LSAMP], f16, tag="dd")
                return X0P, dd0, dd1

            def front_chunk(state, c7):
                """Sampling chain for one 510-column l-chunk; contiguous ops."""
                X0P, dd0, dd1 = state
                n = min(CC, LSAMP - c7 * CC)
                sl = slice(c7 * CC, c7 * CC + n)
                qp = qpsum.tile([NP, CC], f32, tag="qps")
                nc.tensor.matmul(qp[:, :n], wr2_sb[:], X0P[:, sl],
                                 start=True, stop=True)
                ep = qpsum.tile([NP, CC], f32, tag="qps")
                nc.tensor.matmul(ep[:, :n], emat_sb[:], X0P[:, sl],
                                 start=True, stop=True)
                em = qpsum.tile([NP, CC], f32, tag="qps")
                nc.tensor.matmul(em[:, :n], mmatn_sb[:], X0P[:, sl],
                                 start=True, stop=True)
                QSP = qsp.tile([NP, CC], f16, tag="qs")
                nc.scalar.activation(QSP[:, :n], qp[:, :n], Act.Relu,
                                     bias=offb2_sb[:], scale=1.0)
                QSM = qsp.tile([NP, CC], f16, tag="qs")
                nc.scalar.activation(QSM[:, :n], qp[:, :n], Act.Relu,
                                     bias=negoffb2_sb[:], scale=-1.0)
                mA = emp.tile([NP, CC], f16, tag="em")
                nc.vector.tensor_mul(mA[:, :n], ep[:, :n], QSP[:, :n])
                mB = emp.tile([NP, CC], f16, tag="em")
                nc.vector.tensor_mul(mB[:, :n], em[:, :n], QSM[:, :n])
                S = emp.tile([NP, CC], f16, tag="em")
                nc.gpsimd.tensor_add(S[:, :n], mA[:, :n], mB[:, :n])
                nc.gpsimd.tensor_add(dd0[:, sl], X0P[0:K, sl], S[0:K, :n])
                eng1 = nc.vector if c7 % 2 else nc.gpsimd
                eng1.tensor_add(dd1[:, sl], X0P[R1:NP, sl], S[R1:NP, :n])

            SCATTER = {0: nc.scalar, 1: nc.vector}

            def fmm(fp_slice, dd, t0, s_lo, n):
                a = (STRIDE * t0) // K
                t0C = slice(t0 * C_OUT, (t0 + 1) * C_OUT)
                a0 = NSMAX * a + s_lo
                b = a + 1
                b0 = s_lo + 1 if b == STRIDE else NSMAX * b + s_lo
                rhsA = dd[0:K, a0:a0 + n]
                rhsB = dd[0:K, b0:b0 + n]
                nc.tensor.matmul(fp_slice, fa_sb[0:K, t0C], rhsA,
                                 start=True, stop=False)
                nc.tensor.matmul(fp_slice, fb_sb[0:K, t0C], rhsB,
                                 start=False, stop=True)

            def final_pair(r, dd, ysb, t0):
                """t0 and t0+1 in one 2-bank psum tile, full s range; the
                evacuation writes (s, t0)-pairs so consecutive stores are
                4-byte adjacent in ysb."""
                n0 = (T_out - 1 - t0) // K + 1
                n1 = (T_out - 1 - (t0 + 1)) // K + 1
                fp = fpsum.tile([C_OUT, 2, 512], f32, tag="fp")
                fmm(fp[:, 0, :n0], dd, t0, 0, n0)
                fmm(fp[:, 1, :n1], dd, t0 + 1, 0, n1)
                yv2 = ysb[:, t0:t0 + K * n1].rearrange(
                    "p (s q) -> p s q", q=K)[:, :, 0:2]
                sv = fp[:, :, :n1].rearrange("p t s -> p s t")
                ecopy(SCATTER[(r + t0 // 2) % 2], yv2, sv)
                if n0 > n1:
                    yt = ysb[:, t0 + K * n1:t0 + K * n1 + 1]
                    ecopy(SCATTER[(r + t0 // 2 + 1) % 2], yt, fp[:, 0, n1:n0])

            def final_last(r, dd, ysb):
                t0 = K - 1
                n = (T_out - 1 - t0) // K + 1
                fp = fpsum.tile([C_OUT, 2, 512], f32, tag="fp")
                fmm(fp[:, 0, :n], dd, t0, 0, n)
                yv = ysb[:, t0:t0 + K * (n - 1) + 1:K]
                ecopy(SCATTER[r % 2], yv, fp[:, 0, :n])

            def final_rowpass(r, dd, ysb, front=None):
                """front: optional list of thunks interleaved between t0-pairs
                (pair-1 sampling chunks issued during row-0 final)."""
                i = 0
                for t0 in range(0, K - 1, 2):
                    final_pair(r, dd, ysb, t0)
                    if front is not None and t0 % 8 == 0 and i < len(front):
                        front[i]()
                        i += 1
                final_last(r, dd, ysb)
                if front is not None:
                    while i < len(front):
                        front[i]()
                        i += 1

            YSBW = T_out + K - 1

            st0 = alloc_pair()
            st1 = alloc_pair()
            xx0 = load_pair_dma(0)
            load_pair_tp(0, xx0, st0[0])
            xx1 = load_pair_dma(1)
            for c7 in range(NCC):
                front_chunk(st0, c7)
            load_pair_tp(1, xx1, st1[0])

            ysb0 = ysbp.tile([C_OUT, YSBW], f16, tag="ysb")
            front1 = [
                (lambda i=i: front_chunk(st1, i)) for i in range(NCC)
            ]
            final_rowpass(0, st0[1], ysb0, front=front1)
            nc.sync.dma_start(out=y_d[0], in_=ysb0[:, :T_out])

            ysb1 = ysbp.tile([C_OUT, YSBW], f16, tag="ysb")
            final_rowpass(1, st0[2], ysb1)
            nc.sync.dma_start(out=y_d[1], in_=ysb1[:, :T_out])

            ysb2 = ysbp.tile([C_OUT, YSBW], f16, tag="ysb")
            final_rowpass(2, st1[1], ysb2)
            nc.sync.dma_start(out=y_d[2], in_=ysb2[:, :T_out])

            ysb3 = ysbp.tile([C_OUT, YSBW], f16, tag="ysb")
            final_rowpass(3, st1[2], ysb3)
            nc.sync.dma_start(out=y_d[3], in_=ysb3[:, :T_out])

    nc.compile()
    return nc


def _host_inputs(x, hz, band, offset_w, offset_b, B_loc, L):
    """Build the per-core input maps."""
    L_out, T_out, NCHUNK, LPAD, XLEN = _derive(L)
    filt = _host_filters(hz, band)
    f128 = _host_f128(filt, L).astype(np.float16)
    fa = np.ascontiguousarray(f128[0:K])
    fb = np.ascontiguousarray(f128[R1:R1 + K])
    wr = offset_w[:, 0, :].T.astype(np.float32)  # [k_in, k_out]
    wr2 = np.zeros((NP, NP), np.float32)
    wr2[0:K, 0:K] = wr
    wr2[R1:NP, R1:NP] = wr
    emat, mmatn = _host_shift_mats()
    offb2 = np.zeros((NP, 1), np.float32)
    offb2[0:K, 0] = offset_b.astype(np.float32)
    offb2[R1:NP, 0] = offset_b.astype(np.float32)
    negoffb2 = -offb2
    ident = np.eye(128, dtype=np.float16)

    B = x.shape[0]
    xpad = np.zeros((B, XLEN), np.float16)
    xpad[:, 0:L] = x.astype(np.float16)

    n_cores = B // B_loc
    in_maps = []
    for i in range(n_cores):
        in_maps.append({
            "x": np.ascontiguousarray(xpad[i * B_loc:(i + 1) * B_loc]),
            "wr2": wr2.astype(np.float16),
            "emat": emat.astype(np.float16),
            "mmatn": mmatn.astype(np.float16),
            "offb2": offb2,
            "negoffb2": negoffb2,
            "fa": fa,
            "fb": fb,
            "ident": ident,
        })
    return in_maps


_CACHED = {}


def _get_program():
    key = (B_LOC, L_FULL)
    if key not in _CACHED:
        _CACHED[key] = build_program(B_LOC, L_FULL)
    return _CACHED[key]


def kernel(x, hz, band, offset_w, offset_b):
    from concourse.bass_utils import run_bass_kernel_spmd

    x = np.asarray(x, dtype=np.float32)
    hz = np.asarray(hz, dtype=np.float32)
    band = np.asarray(band, dtype=np.float32)
    offset_w = np.asarray(offset_w, dtype=np.float32)
    offset_b = np.asarray(offset_b, dtype=np.float32)

    nc = _get_program()
    in_maps = _host_inputs(x, hz, band, offset_w, offset_b, B_LOC, L_FULL)
    res = run_bass_kernel_spmd(nc, in_maps, list(range(N_CORES)))
    outs = [res.results[i]["y"] for i in range(N_CORES)]
    return np.concatenate(outs, axis=0).astype(np.float32)


# revision 10
# speedup vs baseline: 1.5687x; 1.1731x over previous
"""Trainium2 Bass kernel for DeformableSincConv1d (v3, matmul-shift design).

Data parallel over batch: 4 rows/core on 8 cores. Per core, per batch-pair:
  1. Windowed im2col load (fp16): xx[p, c, j] = x[1280c + 10p + j]; row 0 of
     the pair at cols 0..52, row 1 at cols 64..116 (junk cols 53..63 zeroed).
  2. One PE transpose per 128-l chunk over cols 0..116 -> psum [117, 128];
     a single Act/DVE copy evacuates psum[0:115] straight into X0P — no
     SBUF->SBUF DMA at all (x is laid out with no +1 pad so j aligns).
  3. Sampling, all partition shifts as matmuls: qp = wr2^T X0P (offset conv),
     ep = Emat^T X0P (= X0P[j+1]-X0P[j], col 50 zero), em = Mneg^T X0P
     (= X0P[j-1]-X0P[j], col 0 zero). Act computes o+ = relu(qp + b) and
     o- = relu(-qp - b); deformed = X0P + o+*ep + o-*em exactly reproduces
     the mask/select lerp (offsets |o| < 1).  dd tiles are l-major [51, 3200]
     so every element-wise op is contiguous.
  4. Final conv: stacked rotated-filter decomposition, A-half rows 0..50 and
     B-half rows 64..114 of f128; per t0 two matmuls accumulate in psum
     (rhs = dd strided plane views l = a + 10s and l = a+1 + 10s), so the
     dd[51:102] plane-rotation copies are gone. Psum evacuated with the
     (s, t0)-pair interleaved scatter into fp16 ysb; y written to HBM fp16.
"""

import sys

import numpy as np

if "/opt/trn_rl_repo" not in sys.path:
    sys.path.insert(0, "/opt/trn_rl_repo")

SR = 16000
C_OUT = 80
K = 51
STRIDE = 10
HALF = (K - 1) // 2

B_FULL = 32
N_CORES = 8
B_LOC = B_FULL // N_CORES
L_FULL = 32000

R1 = 64          # partition base of second row in pair tiles
NP = R1 + K      # 115 rows in pair tiles


def _derive(L):
    L_out = (L - K) // STRIDE + 1
    T_out = (L_out * K - K) // STRIDE + 1
    NCHUNK = (L_out + 127) // 128
    LPAD = NCHUNK * 128
    XLEN = 10 * LPAD + 48
    return L_out, T_out, NCHUNK, LPAD, XLEN


def _host_filters(hz, band):
    hzc = np.clip(hz.astype(np.float32), 0.0, SR / 2).astype(np.float32)
    bandc = np.clip(band.astype(np.float32), 3.0, SR / 2).astype(np.float32)
    t_right = (np.arange(1, HALF + 1, dtype=np.float32) / np.float32(SR)).astype(np.float32)
    low = (hzc - bandc / 2).astype(np.float32)
    high = (hzc + bandc / 2).astype(np.float32)

    def sinc(t):
        ts = np.where(t == 0, np.float32(1.0), t)
        return np.where(t == 0, np.float32(1.0), np.sin(ts) / ts).astype(np.float32)

    a1 = (2 * high).astype(np.float32)
    a2 = (2 * low).astype(np.float32)
    bp_left = (a1 * sinc(a1 * t_right) - a2 * sinc(a2 * t_right)).astype(np.float32)
    bp = np.concatenate([bp_left, np.ones((C_OUT, 1), np.float32), bp_left[:, ::-1]], axis=1)
    return (bp / (2 * bandc)).astype(np.float32)  # [C_OUT, K]


def _host_f128(filt, L):
    """Stacked rotated filter matrices [128, K*C_OUT]; rows 0..50 = A-half
    (column offset a), rows 64..114 = B-half (column offset a+1)."""
    L_out, T_out, _, LPAD, _ = _derive(L)
    F = np.zeros((128, K, C_OUT), np.float32)
    for t0 in range(K):
        a = (STRIDE * t0) // K
        ns = (T_out - 1 - t0) // K + 1
        for k2 in range(K):
            kstar = (k2 + STRIDE * t0) % K
            lstar = (STRIDE * t0 + k2) // K
            if lstar == a:
                F[kstar, t0, :] = filt[:, k2]
            else:
                assert lstar == a + 1
                F[R1 + kstar, t0, :] = filt[:, k2]
        assert a + 1 + STRIDE * (ns - 1) <= L_out - 1
    return F.reshape(128, K * C_OUT)


def _host_shift_mats():
    """emat: col j -> X0P[j+1]-X0P[j] (j<50), col 50 zero.
    mmatn: col j -> X0P[j-1]-X0P[j] (j>0), col 0 zero.  Block-diag at 0, R1."""
    emat = np.zeros((NP, NP), np.float32)
    mmatn = np.zeros((NP, NP), np.float32)
    for base in (0, R1):
        for j in range(K - 1):
            emat[base + j + 1, base + j] = 1.0
            emat[base + j, base + j] = -1.0
        for j in range(1, K):
            mmatn[base + j - 1, base + j] = 1.0
            mmatn[base + j, base + j] = -1.0
    return emat, mmatn


def build_program(B_loc=B_LOC, L=L_FULL, debug=False):
    import concourse.bacc as bacc
    import concourse.tile as tile
    from concourse import bass, mybir

    f32 = mybir.dt.float32
    f16 = mybir.dt.float16
    Act = mybir.ActivationFunctionType

    L_out, T_out, NCHUNK, LPAD, XLEN = _derive(L)
    NSMAX = (T_out - 1) // K + 1    # 320
    LSAMP = STRIDE * NSMAX          # sampled deformed region (3200)
    assert LSAMP <= LPAD
    NG = (NCHUNK + 7) // 8          # transpose psum groups of 8 chunks
    CC = 510
    NCC = (LSAMP + CC - 1) // CC
    n_pairs = B_loc // 2
    assert n_pairs == 2

    nc = bacc.Bacc("TRN2", target_bir_lowering=False, debug=debug)

    x_d = nc.dram_tensor("x", [B_loc, XLEN], f16, kind="ExternalInput")
    wr2_d = nc.dram_tensor("wr2", [NP, NP], f16, kind="ExternalInput")
    emat_d = nc.dram_tensor("emat", [NP, NP], f16, kind="ExternalInput")
    mmatn_d = nc.dram_tensor("mmatn", [NP, NP], f16, kind="ExternalInput")
    offb2_d = nc.dram_tensor("offb2", [NP, 1], f32, kind="ExternalInput")
    negoffb2_d = nc.dram_tensor("negoffb2", [NP, 1], f32, kind="ExternalInput")
    fa_d = nc.dram_tensor("fa", [K, K * C_OUT], f16, kind="ExternalInput")
    fb_d = nc.dram_tensor("fb", [K, K * C_OUT], f16, kind="ExternalInput")
    ident_d = nc.dram_tensor("ident", [128, 128], f16, kind="ExternalInput")
    y_d = nc.dram_tensor("y", [B_loc, C_OUT, T_out], f16, kind="ExternalOutput")

    xap = x_d[:]

    with tile.TileContext(nc) as tc:
        with (
            tc.tile_pool(name="consts", bufs=1) as consts,
            tc.tile_pool(name="xxp", bufs=2) as xxp,
            tc.tile_pool(name="x0p", bufs=2) as x0p,
            tc.tile_pool(name="qsp", bufs=4) as qsp,
            tc.tile_pool(name="emp", bufs=6) as emp,
            tc.tile_pool(name="ddp", bufs=4) as ddp,
            tc.tile_pool(name="ysbp", bufs=2) as ysbp,
            tc.tile_pool(name="tpsum", bufs=1, space="PSUM") as tpsum,
            tc.tile_pool(name="qpsum", bufs=3, space="PSUM") as qpsum,
            tc.tile_pool(name="fpsum", bufs=4, space="PSUM") as fpsum,
        ):
            wr2_sb = consts.tile([NP, NP], f16)
            nc.sync.dma_start(out=wr2_sb[:], in_=wr2_d[:])
            emat_sb = consts.tile([NP, NP], f16)
            nc.sync.dma_start(out=emat_sb[:], in_=emat_d[:])
            mmatn_sb = consts.tile([NP, NP], f16)
            nc.sync.dma_start(out=mmatn_sb[:], in_=mmatn_d[:])
            offb2_sb = consts.tile([NP, 1], f32)
            nc.sync.dma_start(out=offb2_sb[:], in_=offb2_d[:])
            negoffb2_sb = consts.tile([NP, 1], f32)
            nc.sync.dma_start(out=negoffb2_sb[:], in_=negoffb2_d[:])
            fa_sb = consts.tile([K, K * C_OUT], f16)
            nc.sync.dma_start(out=fa_sb[:], in_=fa_d[:])
            fb_sb = consts.tile([K, K * C_OUT], f16)
            nc.sync.dma_start(out=fb_sb[:], in_=fb_d[:])
            ident_sb = consts.tile([128, 128], f16)
            nc.sync.dma_start(out=ident_sb[:], in_=ident_d[:])

            def ecopy(eng, dst, src):
                if eng is nc.scalar:
                    eng.copy(dst, src)
                else:
                    eng.tensor_copy(dst, src)

            def load_pair_dma(p):
                """im2col in polyphase window order: xx partition pi%128 of
                chunk pi//128 holds the window for pi = 320*r + s, i.e.
                l = 10*s + r, so downstream X0P/dd are plane-major and every
                op (including final-conv rhs) is contiguous."""
                xx = xxp.tile([128, NCHUNK, 128], f16, tag="xx")
                nc.vector.memset(xx[:, :, 53:64], 0.0)
                for c in range(NCHUNK):
                    for ri in range(2):
                        col0 = R1 * ri
                        row_off = (2 * p + ri) * XLEN
                        pi = 128 * c
                        while pi < 128 * (c + 1):
                            r, s0 = divmod(pi, NSMAX)
                            npart = min(128 * (c + 1) - pi, NSMAX - s0)
                            in_ap = bass.AP(
                                tensor=xap.tensor,
                                offset=row_off + 100 * s0 + 10 * r,
                                ap=[[100, npart], [1, 53]],
                            )
                            p0 = pi - 128 * c
                            nc.sync.dma_start(
                                out=xx[p0:p0 + npart, c, col0:col0 + 53],
                                in_=in_ap)
                            pi += npart
                return xx

            def load_pair_tp(p, xx, X0P):
                for g in range(NG):
                    n = min(8, NCHUNK - 8 * g)
                    pt = tpsum.tile([117, 1024], f16, tag="pt")
                    for c in range(n):
                        nc.tensor.transpose(pt[:, c * 128:(c + 1) * 128],
                                            xx[:, 8 * g + c, 0:117], ident_sb[:])
                    lo = g * 1024
                    nw = min(n * 128, LSAMP - lo)
                    if nw > 0:
                        ecopy([nc.vector, nc.scalar][(p + g) % 2],
                              X0P[:, lo:lo + nw], pt[0:NP, :nw])

            def alloc_pair():
                X0P = x0p.tile([NP, LPAD], f16)
                dd0 = ddp.tile([K, LSAMP], f16, tag="dd")
                dd1 = ddp.tile([K, LSAMP], f16, tag="dd")
                return X0P, dd0, dd1

            def front_chunk(state, c7):
                """Sampling chain for one 510-column l-chunk; contiguous ops."""
                X0P, dd0, dd1 = state
                n = min(CC, LSAMP - c7 * CC)
                sl = slice(c7 * CC, c7 * CC + n)
                qp = qpsum.tile([NP, CC], f32, tag="qps")
                nc.tensor.matmul(qp[:, :n], wr2_sb[:], X0P[:, sl],
                                 start=True, stop=True)
                ep = qpsum.tile([NP, CC], f32, tag="qps")
                nc.tensor.matmul(ep[:, :n], emat_sb[:], X0P[:, sl],
                                 start=True, stop=True)
                em = qpsum.tile([NP, CC], f32, tag="qps")
                nc.tensor.matmul(em[:, :n], mmatn_sb[:], X0P[:, sl],
                                 start=True, stop=True)
                QSP = qsp.tile([NP, CC], f16, tag="qs")
                nc.scalar.activation(QSP[:, :n], qp[:, :n], Act.Relu,
                                     bias=offb2_sb[:], scale=1.0)
                QSM = qsp.tile([NP, CC], f16, tag="qs")
                nc.scalar.activation(QSM[:, :n], qp[:, :n], Act.Relu,
                                     bias=negoffb2_sb[:], scale=-1.0)
                mA = emp.tile([NP, CC], f16, tag="em")
                nc.vector.tensor_mul(mA[:, :n], ep[:, :n], QSP[:, :n])
                mB = emp.tile([NP, CC], f16, tag="em")
                nc.vector.tensor_mul(mB[:, :n], em[:, :n], QSM[:, :n])
                S = emp.tile([NP, CC], f16, tag="em")
                nc.gpsimd.tensor_add(S[:, :n], mA[:, :n], mB[:, :n])
                nc.gpsimd.tensor_add(dd0[:, sl], X0P[0:K, sl], S[0:K, :n])
                eng1 = nc.vector if c7 % 2 else nc.gpsimd
                eng1.tensor_add(dd1[:, sl], X0P[R1:NP, sl], S[R1:NP, :n])

            SCATTER = {0: nc.scalar, 1: nc.vector}

            def fmm(fp_slice, dd, t0, s_lo, n):
                a = (STRIDE * t0) // K
                t0C = slice(t0 * C_OUT, (t0 + 1) * C_OUT)
                a0 = NSMAX * a + s_lo
                b = a + 1
                b0 = s_lo + 1 if b == STRIDE else NSMAX * b + s_lo
                rhsA = dd[0:K, a0:a0 + n]
                rhsB = dd[0:K, b0:b0 + n]
                nc.tensor.matmul(fp_slice, fa_sb[0:K, t0C], rhsA,
                                 start=True, stop=False)
                nc.tensor.matmul(fp_slice, fb_sb[0:K, t0C], rhsB,
                                 start=False, stop=True)

            HH = NSMAX // 2   # s-half size (160)

            def final_pair_h(r, dd, ysb, t0, h):
                """t0 and t0+1 in one 1-bank psum tile per s-half; the
                evacuation writes (s, t0)-pairs so consecutive stores are
                4-byte adjacent in ysb."""
                ns0 = (T_out - 1 - t0) // K + 1
                ns1 = (T_out - 1 - (t0 + 1)) // K + 1
                s_lo = h * HH
                n0 = min(ns0 - s_lo, HH)
                n1 = min(ns1 - s_lo, HH)
                fp = fpsum.tile([C_OUT, 2, 256], f32, tag="fp")
                fmm(fp[:, 0, :n0], dd, t0, s_lo, n0)
                fmm(fp[:, 1, :n1], dd, t0 + 1, s_lo, n1)
                base = t0 + K * s_lo
                yv2 = ysb[:, base:base + K * n1].rearrange(
                    "p (s q) -> p s q", q=K)[:, :, 0:2]
                sv = fp[:, :, :n1].rearrange("p t s -> p s t")
                ecopy(SCATTER[(r + t0 // 2 + h) % 2], yv2, sv)
                if n0 > n1:
                    yt = ysb[:, base + K * n1:base + K * n1 + 1]
                    ecopy(SCATTER[(r + t0 // 2 + h + 1) % 2], yt,
                          fp[:, 0, n1:n0])

            def final_last(r, dd, ysb):
                t0 = K - 1
                ns = (T_out - 1 - t0) // K + 1
                for h in range(2):
                    s_lo = h * HH
                    n = min(ns - s_lo, HH)
                    fp = fpsum.tile([C_OUT, 2, 256], f32, tag="fp")
                    fmm(fp[:, 0, :n], dd, t0, s_lo, n)
                    yv = ysb[:, t0 + K * s_lo:t0 + K * (s_lo + n - 1) + 1:K]
                    ecopy(SCATTER[(r + h) % 2], yv, fp[:, 0, :n])

            def final_rowpass(r, dd, ysb, front=None):
                """front: optional list of thunks interleaved between t0-pairs
                (pair-1 sampling chunks issued during row-0 final)."""
                i = 0
                for t0 in range(0, K - 1, 2):
                    for h in range(2):
                        final_pair_h(r, dd, ysb, t0, h)
                    if front is not None and t0 % 8 == 0 and i < len(front):
                        front[i]()
                        i += 1
                final_last(r, dd, ysb)
                if front is not None:
                    while i < len(front):
                        front[i]()
                        i += 1

            YSBW = T_out + K - 1

            st0 = alloc_pair()
            st1 = alloc_pair()
            xx0 = load_pair_dma(0)
            load_pair_tp(0, xx0, st0[0])
            xx1 = load_pair_dma(1)
            for c7 in range(NCC):
                front_chunk(st0, c7)
            load_pair_tp(1, xx1, st1[0])

            ysb0 = ysbp.tile([C_OUT, YSBW], f16, tag="ysb")
            front1 = [
                (lambda i=i: front_chunk(st1, i)) for i in range(NCC)
            ]
            final_rowpass(0, st0[1], ysb0, front=front1)
            nc.sync.dma_start(out=y_d[0], in_=ysb0[:, :T_out])

            ysb1 = ysbp.tile([C_OUT, YSBW], f16, tag="ysb")
            final_rowpass(1, st0[2], ysb1)
            nc.sync.dma_start(out=y_d[1], in_=ysb1[:, :T_out])

            ysb2 = ysbp.tile([C_OUT, YSBW], f16, tag="ysb")
            final_rowpass(2, st1[1], ysb2)
            nc.sync.dma_start(out=y_d[2], in_=ysb2[:, :T_out])

            ysb3 = ysbp.tile([C_OUT, YSBW], f16, tag="ysb")
            final_rowpass(3, st1[2], ysb3)
            nc.sync.dma_start(out=y_d[3], in_=ysb3[:, :T_out])

    nc.compile()
    return nc


def _host_inputs(x, hz, band, offset_w, offset_b, B_loc, L):
    """Build the per-core input maps."""
    L_out, T_out, NCHUNK, LPAD, XLEN = _derive(L)
    filt = _host_filters(hz, band)
    f128 = _host_f128(filt, L).astype(np.float16)
    fa = np.ascontiguousarray(f128[0:K])
    fb = np.ascontiguousarray(f128[R1:R1 + K])
    wr = offset_w[:, 0, :].T.astype(np.float32)  # [k_in, k_out]
    wr2 = np.zeros((NP, NP), np.float32)
    wr2[0:K, 0:K] = wr
    wr2[R1:NP, R1:NP] = wr
    emat, mmatn = _host_shift_mats()
    offb2 = np.zeros((NP, 1), np.float32)
    offb2[0:K, 0] = offset_b.astype(np.float32)
    offb2[R1:NP, 0] = offset_b.astype(np.float32)
    negoffb2 = -offb2
    ident = np.eye(128, dtype=np.float16)

    B = x.shape[0]
    xpad = np.zeros((B, XLEN), np.float16)
    xpad[:, 0:L] = x.astype(np.float16)

    n_cores = B // B_loc
    in_maps = []
    for i in range(n_cores):
        in_maps.append({
            "x": np.ascontiguousarray(xpad[i * B_loc:(i + 1) * B_loc]),
            "wr2": wr2.astype(np.float16),
            "emat": emat.astype(np.float16),
            "mmatn": mmatn.astype(np.float16),
            "offb2": offb2,
            "negoffb2": negoffb2,
            "fa": fa,
            "fb": fb,
            "ident": ident,
        })
    return in_maps


_CACHED = {}


def _get_program():
    key = (B_LOC, L_FULL)
    if key not in _CACHED:
        _CACHED[key] = build_program(B_LOC, L_FULL)
    return _CACHED[key]


def kernel(x, hz, band, offset_w, offset_b):
    from concourse.bass_utils import run_bass_kernel_spmd

    x = np.asarray(x, dtype=np.float32)
    hz = np.asarray(hz, dtype=np.float32)
    band = np.asarray(band, dtype=np.float32)
    offset_w = np.asarray(offset_w, dtype=np.float32)
    offset_b = np.asarray(offset_b, dtype=np.float32)

    nc = _get_program()
    in_maps = _host_inputs(x, hz, band, offset_w, offset_b, B_LOC, L_FULL)
    res = run_bass_kernel_spmd(nc, in_maps, list(range(N_CORES)))
    outs = [res.results[i]["y"] for i in range(N_CORES)]
    return np.concatenate(outs, axis=0).astype(np.float32)


# revision 12
# speedup vs baseline: 1.5902x; 1.0137x over previous
"""Trainium2 Bass kernel for DeformableSincConv1d (v3, matmul-shift design).

Data parallel over batch: 4 rows/core on 8 cores. Per core, per batch-pair:
  1. Windowed im2col load (fp16): xx[p, c, j] = x[1280c + 10p + j]; row 0 of
     the pair at cols 0..52, row 1 at cols 64..116 (junk cols 53..63 zeroed).
  2. One PE transpose per 128-l chunk over cols 0..116 -> psum [117, 128];
     a single Act/DVE copy evacuates psum[0:115] straight into X0P — no
     SBUF->SBUF DMA at all (x is laid out with no +1 pad so j aligns).
  3. Sampling, all partition shifts as matmuls: qp = wr2^T X0P (offset conv),
     ep = Emat^T X0P (= X0P[j+1]-X0P[j], col 50 zero), em = Mneg^T X0P
     (= X0P[j-1]-X0P[j], col 0 zero). Act computes o+ = relu(qp + b) and
     o- = relu(-qp - b); deformed = X0P + o+*ep + o-*em exactly reproduces
     the mask/select lerp (offsets |o| < 1).  dd tiles are l-major [51, 3200]
     so every element-wise op is contiguous.
  4. Final conv: stacked rotated-filter decomposition, A-half rows 0..50 and
     B-half rows 64..114 of f128; per t0 two matmuls accumulate in psum
     (rhs = dd strided plane views l = a + 10s and l = a+1 + 10s), so the
     dd[51:102] plane-rotation copies are gone. Psum evacuated with the
     (s, t0)-pair interleaved scatter into fp16 ysb; y written to HBM fp16.
"""

import sys

import numpy as np

if "/opt/trn_rl_repo" not in sys.path:
    sys.path.insert(0, "/opt/trn_rl_repo")

SR = 16000
C_OUT = 80
K = 51
STRIDE = 10
HALF = (K - 1) // 2

B_FULL = 32
N_CORES = 8
B_LOC = B_FULL // N_CORES
L_FULL = 32000

R1 = 64          # partition base of second row in pair tiles
NP = R1 + K      # 115 rows in pair tiles


def _derive(L):
    L_out = (L - K) // STRIDE + 1
    T_out = (L_out * K - K) // STRIDE + 1
    NCHUNK = (L_out + 127) // 128
    LPAD = NCHUNK * 128
    XLEN = 10 * LPAD + 48
    return L_out, T_out, NCHUNK, LPAD, XLEN


def _host_filters(hz, band):
    hzc = np.clip(hz.astype(np.float32), 0.0, SR / 2).astype(np.float32)
    bandc = np.clip(band.astype(np.float32), 3.0, SR / 2).astype(np.float32)
    t_right = (np.arange(1, HALF + 1, dtype=np.float32) / np.float32(SR)).astype(np.float32)
    low = (hzc - bandc / 2).astype(np.float32)
    high = (hzc + bandc / 2).astype(np.float32)

    def sinc(t):
        ts = np.where(t == 0, np.float32(1.0), t)
        return np.where(t == 0, np.float32(1.0), np.sin(ts) / ts).astype(np.float32)

    a1 = (2 * high).astype(np.float32)
    a2 = (2 * low).astype(np.float32)
    bp_left = (a1 * sinc(a1 * t_right) - a2 * sinc(a2 * t_right)).astype(np.float32)
    bp = np.concatenate([bp_left, np.ones((C_OUT, 1), np.float32), bp_left[:, ::-1]], axis=1)
    return (bp / (2 * bandc)).astype(np.float32)  # [C_OUT, K]


def _host_f128(filt, L):
    """Stacked rotated filter matrices [128, K*C_OUT]; rows 0..50 = A-half
    (column offset a), rows 64..114 = B-half (column offset a+1)."""
    L_out, T_out, _, LPAD, _ = _derive(L)
    F = np.zeros((128, K, C_OUT), np.float32)
    for t0 in range(K):
        a = (STRIDE * t0) // K
        ns = (T_out - 1 - t0) // K + 1
        for k2 in range(K):
            kstar = (k2 + STRIDE * t0) % K
            lstar = (STRIDE * t0 + k2) // K
            if lstar == a:
                F[kstar, t0, :] = filt[:, k2]
            else:
                assert lstar == a + 1
                F[R1 + kstar, t0, :] = filt[:, k2]
        assert a + 1 + STRIDE * (ns - 1) <= L_out - 1
    return F.reshape(128, K * C_OUT)


def _host_shift_mats():
    """emat: col j -> X0P[j+1]-X0P[j] (j<50), col 50 zero.
    mmatn: col j -> X0P[j-1]-X0P[j] (j>0), col 0 zero.  Block-diag at 0, R1."""
    emat = np.zeros((NP, NP), np.float32)
    mmatn = np.zeros((NP, NP), np.float32)
    for base in (0, R1):
        for j in range(K - 1):
            emat[base + j + 1, base + j] = 1.0
            emat[base + j, base + j] = -1.0
        for j in range(1, K):
            mmatn[base + j - 1, base + j] = 1.0
            mmatn[base + j, base + j] = -1.0
    return emat, mmatn


def build_program(B_loc=B_LOC, L=L_FULL, debug=False):
    import concourse.bacc as bacc
    import concourse.tile as tile
    from concourse import bass, mybir

    f32 = mybir.dt.float32
    f16 = mybir.dt.float16
    Act = mybir.ActivationFunctionType

    L_out, T_out, NCHUNK, LPAD, XLEN = _derive(L)
    NSMAX = (T_out - 1) // K + 1    # 320 real s slots per plane
    SPL = 384                       # padded plane pitch (3 x 128 chunks)
    LSAMP = STRIDE * SPL            # padded polyphase extent (3840)
    NCHUNK = LSAMP // 128           # 30
    NG = (NCHUNK + 7) // 8          # transpose psum groups of 8 chunks
    CC = 480
    NCC = LSAMP // CC               # 8
    n_pairs = B_loc // 2
    assert n_pairs == 2

    nc = bacc.Bacc("TRN2", target_bir_lowering=False, debug=debug)

    x_d = nc.dram_tensor("x", [B_loc, XLEN], f16, kind="ExternalInput")
    wr2_d = nc.dram_tensor("wr2", [NP, NP], f16, kind="ExternalInput")
    emat_d = nc.dram_tensor("emat", [NP, NP], f16, kind="ExternalInput")
    mmatn_d = nc.dram_tensor("mmatn", [NP, NP], f16, kind="ExternalInput")
    offb2_d = nc.dram_tensor("offb2", [NP, 1], f32, kind="ExternalInput")
    negoffb2_d = nc.dram_tensor("negoffb2", [NP, 1], f32, kind="ExternalInput")
    fa_d = nc.dram_tensor("fa", [K, K * C_OUT], f16, kind="ExternalInput")
    fb_d = nc.dram_tensor("fb", [K, K * C_OUT], f16, kind="ExternalInput")
    ident_d = nc.dram_tensor("ident", [128, 128], f16, kind="ExternalInput")
    y_d = nc.dram_tensor("y", [B_loc, C_OUT, T_out], f16, kind="ExternalOutput")

    xap = x_d[:]

    with tile.TileContext(nc) as tc:
        with (
            tc.tile_pool(name="consts", bufs=1) as consts,
            tc.tile_pool(name="xxp", bufs=2) as xxp,
            tc.tile_pool(name="x0p", bufs=2) as x0p,
            tc.tile_pool(name="qsp", bufs=4) as qsp,
            tc.tile_pool(name="emp", bufs=6) as emp,
            tc.tile_pool(name="ddp", bufs=4) as ddp,
            tc.tile_pool(name="ysbp", bufs=2) as ysbp,
            tc.tile_pool(name="tpsum", bufs=1, space="PSUM") as tpsum,
            tc.tile_pool(name="qpsum", bufs=3, space="PSUM") as qpsum,
            tc.tile_pool(name="fpsum", bufs=4, space="PSUM") as fpsum,
        ):
            wr2_sb = consts.tile([NP, NP], f16)
            nc.sync.dma_start(out=wr2_sb[:], in_=wr2_d[:])
            emat_sb = consts.tile([NP, NP], f16)
            nc.sync.dma_start(out=emat_sb[:], in_=emat_d[:])
            mmatn_sb = consts.tile([NP, NP], f16)
            nc.sync.dma_start(out=mmatn_sb[:], in_=mmatn_d[:])
            offb2_sb = consts.tile([NP, 1], f32)
            nc.sync.dma_start(out=offb2_sb[:], in_=offb2_d[:])
            negoffb2_sb = consts.tile([NP, 1], f32)
            nc.sync.dma_start(out=negoffb2_sb[:], in_=negoffb2_d[:])
            fa_sb = consts.tile([K, K * C_OUT], f16)
            nc.sync.dma_start(out=fa_sb[:], in_=fa_d[:])
            fb_sb = consts.tile([K, K * C_OUT], f16)
            nc.sync.dma_start(out=fb_sb[:], in_=fb_d[:])
            ident_sb = consts.tile([128, 128], f16)
            nc.sync.dma_start(out=ident_sb[:], in_=ident_d[:])

            def ecopy(eng, dst, src):
                if eng is nc.scalar:
                    eng.copy(dst, src)
                else:
                    eng.tensor_copy(dst, src)

            def load_pair_dma(p):
                """im2col in padded polyphase order: xx partition pi%128 of
                chunk pi//128 holds the window for pi = 384*r + s (s < 320),
                i.e. l = 10*s + r, so downstream X0P/dd are plane-major and
                every op (including final-conv rhs) is contiguous.  384 = 3
                chunks per plane makes the dst AP rectangular: one dma_start
                covers s 0..255 ([128 part, 2 chunks]) and one s 256..319."""
                xx = xxp.tile([128, NCHUNK, 128], f16, tag="xx")
                nc.vector.memset(xx[:, :, 53:64], 0.0)
                # pad windows (s >= 320): partitions 64.. of every 3rd chunk
                nc.vector.memset(xx[64:128, 2:NCHUNK:3, :], 0.0)
                for r in range(STRIDE):
                    for ri in range(2):
                        col0 = R1 * ri
                        row_off = (2 * p + ri) * XLEN + 10 * r
                        c0 = 3 * r
                        nc.sync.dma_start(
                            out=xx[:, c0:c0 + 2, col0:col0 + 53],
                            in_=bass.AP(tensor=xap.tensor,
                                        offset=row_off,
                                        ap=[[100, 128], [12800, 2], [1, 53]]))
                        nc.sync.dma_start(
                            out=xx[0:64, c0 + 2, col0:col0 + 53],
                            in_=bass.AP(tensor=xap.tensor,
                                        offset=row_off + 100 * 256,
                                        ap=[[100, 64], [1, 53]]))
                return xx

            def load_pair_tp(p, xx, X0P):
                for g in range(NG):
                    n = min(8, NCHUNK - 8 * g)
                    pt = tpsum.tile([117, 1024], f16, tag="pt")
                    for c in range(n):
                        nc.tensor.transpose(pt[:, c * 128:(c + 1) * 128],
                                            xx[:, 8 * g + c, 0:117], ident_sb[:])
                    lo = g * 1024
                    nw = min(n * 128, LSAMP - lo)
                    if nw > 0:
                        ecopy([nc.vector, nc.scalar][(p + g) % 2],
                              X0P[:, lo:lo + nw], pt[0:NP, :nw])

            def alloc_pair():
                X0P = x0p.tile([NP, LSAMP], f16)
                dd0 = ddp.tile([K, LSAMP], f16, tag="dd")
                dd1 = ddp.tile([K, LSAMP], f16, tag="dd")
                return X0P, dd0, dd1

            def front_chunk(state, c7):
                """Sampling chain for one 510-column l-chunk; contiguous ops."""
                X0P, dd0, dd1 = state
                n = min(CC, LSAMP - c7 * CC)
                sl = slice(c7 * CC, c7 * CC + n)
                qp = qpsum.tile([NP, CC], f32, tag="qps")
                nc.tensor.matmul(qp[:, :n], wr2_sb[:], X0P[:, sl],
                                 start=True, stop=True)
                ep = qpsum.tile([NP, CC], f32, tag="qps")
                nc.tensor.matmul(ep[:, :n], emat_sb[:], X0P[:, sl],
                                 start=True, stop=True)
                em = qpsum.tile([NP, CC], f32, tag="qps")
                nc.tensor.matmul(em[:, :n], mmatn_sb[:], X0P[:, sl],
                                 start=True, stop=True)
                QSP = qsp.tile([NP, CC], f16, tag="qs")
                nc.scalar.activation(QSP[:, :n], qp[:, :n], Act.Relu,
                                     bias=offb2_sb[:], scale=1.0)
                QSM = qsp.tile([NP, CC], f16, tag="qs")
                nc.scalar.activation(QSM[:, :n], qp[:, :n], Act.Relu,
                                     bias=negoffb2_sb[:], scale=-1.0)
                mA = emp.tile([NP, CC], f16, tag="em")
                nc.vector.tensor_mul(mA[:, :n], ep[:, :n], QSP[:, :n])
                mB = emp.tile([NP, CC], f16, tag="em")
                nc.vector.tensor_mul(mB[:, :n], em[:, :n], QSM[:, :n])
                S = emp.tile([NP, CC], f16, tag="em")
                nc.gpsimd.tensor_add(S[:, :n], mA[:, :n], mB[:, :n])
                nc.gpsimd.tensor_add(dd0[:, sl], X0P[0:K, sl], S[0:K, :n])
                eng1 = nc.vector if c7 % 2 else nc.gpsimd
                eng1.tensor_add(dd1[:, sl], X0P[R1:NP, sl], S[R1:NP, :n])

            SCATTER = {0: nc.scalar, 1: nc.vector}

            def fmm(fp_slice, dd, t0, s_lo, n):
                a = (STRIDE * t0) // K
                t0C = slice(t0 * C_OUT, (t0 + 1) * C_OUT)
                a0 = SPL * a + s_lo
                b = a + 1
                b0 = s_lo + 1 if b == STRIDE else SPL * b + s_lo
                rhsA = dd[0:K, a0:a0 + n]
                rhsB = dd[0:K, b0:b0 + n]
                nc.tensor.matmul(fp_slice, fa_sb[0:K, t0C], rhsA,
                                 start=True, stop=False)
                nc.tensor.matmul(fp_slice, fb_sb[0:K, t0C], rhsB,
                                 start=False, stop=True)

            HH = NSMAX // 2   # s-half size (160)

            def final_trip_h(r, dd, ysb, t0, h):
                """t0, t0+1, t0+2 in one 1-bank psum tile per s-half; the
                evacuation writes (s, t0)-triples so consecutive stores are
                6-byte adjacent in ysb."""
                s_lo = h * HH
                ns = [(T_out - 1 - (t0 + i)) // K + 1 for i in range(3)]
                n = [min(x - s_lo, HH) for x in ns]
                fp = fpsum.tile([C_OUT, 3, HH], f32, tag="fp")
                for i in range(3):
                    fmm(fp[:, i, :n[i]], dd, t0 + i, s_lo, n[i])
                base = t0 + K * s_lo
                n2 = n[2]
                yv3 = ysb[:, base:base + K * n2].rearrange(
                    "p (s q) -> p s q", q=K)[:, :, 0:3]
                sv = fp[:, :, :n2].rearrange("p t s -> p s t")
                ecopy(SCATTER[(r + t0 + h) % 2], yv3, sv)
                if n[1] > n2:
                    yt = ysb[:, base + K * n2:base + K * n2 + 2]
                    ecopy(SCATTER[(r + t0 + h + 1) % 2], yt, fp[:, 0:2, n2])
                if n[0] > n[1]:
                    yt = ysb[:, base + K * n[1]:base + K * n[1] + 1]
                    ecopy(SCATTER[(r + t0 + h) % 2], yt, fp[:, 0, n[1]:n[0]])

            def final_rowpass(r, dd, ysb, front=None):
                """front: optional list of thunks interleaved between t0
                triples (pair-1 sampling chunks issued during row-0 final)."""
                i = 0
                for g in range(K // 3):
                    for h in range(2):
                        final_trip_h(r, dd, ysb, 3 * g, h)
                    if front is not None and g % 2 == 0 and i < len(front):
                        front[i]()
                        i += 1
                if front is not None:
                    while i < len(front):
                        front[i]()
                        i += 1

            YSBW = T_out + K - 1

            st0 = alloc_pair()
            st1 = alloc_pair()
            xx0 = load_pair_dma(0)
            load_pair_tp(0, xx0, st0[0])
            xx1 = load_pair_dma(1)
            for c7 in range(NCC):
                front_chunk(st0, c7)
            load_pair_tp(1, xx1, st1[0])

            ysb0 = ysbp.tile([C_OUT, YSBW], f16, tag="ysb")
            front1 = [
                (lambda i=i: front_chunk(st1, i)) for i in range(NCC)
            ]
            final_rowpass(0, st0[1], ysb0, front=front1)
            nc.sync.dma_start(out=y_d[0], in_=ysb0[:, :T_out])

            ysb1 = ysbp.tile([C_OUT, YSBW], f16, tag="ysb")
            final_rowpass(1, st0[2], ysb1)
            nc.sync.dma_start(out=y_d[1], in_=ysb1[:, :T_out])

            ysb2 = ysbp.tile([C_OUT, YSBW], f16, tag="ysb")
            final_rowpass(2, st1[1], ysb2)
            nc.sync.dma_start(out=y_d[2], in_=ysb2[:, :T_out])

            ysb3 = ysbp.tile([C_OUT, YSBW], f16, tag="ysb")
            final_rowpass(3, st1[2], ysb3)
            nc.sync.dma_start(out=y_d[3], in_=ysb3[:, :T_out])

    nc.compile()
    return nc


def _host_inputs(x, hz, band, offset_w, offset_b, B_loc, L):
    """Build the per-core input maps."""
    L_out, T_out, NCHUNK, LPAD, XLEN = _derive(L)
    filt = _host_filters(hz, band)
    f128 = _host_f128(filt, L).astype(np.float16)
    fa = np.ascontiguousarray(f128[0:K])
    fb = np.ascontiguousarray(f128[R1:R1 + K])
    wr = offset_w[:, 0, :].T.astype(np.float32)  # [k_in, k_out]
    wr2 = np.zeros((NP, NP), np.float32)
    wr2[0:K, 0:K] = wr
    wr2[R1:NP, R1:NP] = wr
    emat, mmatn = _host_shift_mats()
    offb2 = np.zeros((NP, 1), np.float32)
    offb2[0:K, 0] = offset_b.astype(np.float32)
    offb2[R1:NP, 0] = offset_b.astype(np.float32)
    negoffb2 = -offb2
    ident = np.eye(128, dtype=np.float16)

    B = x.shape[0]
    xpad = np.zeros((B, XLEN), np.float16)
    xpad[:, 0:L] = x.astype(np.float16)

    n_cores = B // B_loc
    in_maps = []
    for i in range(n_cores):
        in_maps.append({
            "x": np.ascontiguousarray(xpad[i * B_loc:(i + 1) * B_loc]),
            "wr2": wr2.astype(np.float16),
            "emat": emat.astype(np.float16),
            "mmatn": mmatn.astype(np.float16),
            "offb2": offb2,
            "negoffb2": negoffb2,
            "fa": fa,
            "fb": fb,
            "ident": ident,
        })
    return in_maps


_CACHED = {}


def _get_program():
    key = (B_LOC, L_FULL)
    if key not in _CACHED:
        _CACHED[key] = build_program(B_LOC, L_FULL)
    return _CACHED[key]


def kernel(x, hz, band, offset_w, offset_b):
    from concourse.bass_utils import run_bass_kernel_spmd

    x = np.asarray(x, dtype=np.float32)
    hz = np.asarray(hz, dtype=np.float32)
    band = np.asarray(band, dtype=np.float32)
    offset_w = np.asarray(offset_w, dtype=np.float32)
    offset_b = np.asarray(offset_b, dtype=np.float32)

    nc = _get_program()
    in_maps = _host_inputs(x, hz, band, offset_w, offset_b, B_LOC, L_FULL)
    res = run_bass_kernel_spmd(nc, in_maps, list(range(N_CORES)))
    outs = [res.results[i]["y"] for i in range(N_CORES)]
    return np.concatenate(outs, axis=0).astype(np.float32)
